# revision 15
# baseline (speedup 1.0000x reference)
"""Trainium2 Bass kernel for nn_MultiHeadAttention_70540542870156.

Full MHA: out = softmax(mask(QK^T/D)) V Wo + bo, plus the attention matrix
itself as a second output [H*B, L, L].

Sharding (8 cores): core c handles batch b = c//4 and heads
[4*(c%4), 4*(c%4)+4) — i.e. 1 batch x 4 heads per core (dk slice of 256).
Host pre-transposes q/k/v to [D, L] per batch, slices weights, and converts
the mask to an additive fp8 bias (0 / -240) in [k, q] layout.

Device pipeline per core (all-fp32 matmuls):
  - QKV projections on TensorE; q-hat/k-hat written with both SBUF partition
    halves holding each head's rows (via stride-0 broadcast of the weight
    columns), enabling 2-way row-packed score matmuls.
  - Per (head, q-block of 512): S^T tiles [128k x 512q] via packed matmuls,
    mask bias added by an identity matmul streaming the fp8 bias into PSUM,
    single ScalarE exp pass evacuating PSUM, PV matmul with an appended
    ones-row producing context and softmax denominators, TensorE transposes
    of exp(S^T) to [q, k] layout, and a fused VectorE PSUM-evacuate +
    normalize (tensor_scalar by 1/denom per-partition) before the DMA store.
  - Output projection from a DRAM-staged context, normalized upstream.
"""

import os
import sys

import numpy as np

for _p in ("/opt/trn_rl_repo",):
    if _p not in sys.path and os.path.isdir(_p):
        sys.path.insert(0, _p)

import ml_dtypes

B, L, D, H = 2, 2048, 1024, 16
DK = D // H  # 64
HPC = 4  # heads per core
DKC = HPC * DK  # 256
N_CORES = 8
QB = 512  # q-block (moving free dim)
NQB = L // QB  # 4
NKC = L // 128  # 16 k-chunks
F8 = ml_dtypes.float8_e4m3
MASK_BIAS = -240.0  # exp(x - 240) underflows fp32 to 0 for x ~ 0


def build_bass():
    import concourse.bass as bass
    import concourse.mybir as mybir
    import concourse.tile as tile
    from concourse import bacc
    from concourse.masks import make_identity

    f32 = mybir.dt.float32
    f8 = mybir.dt.float8e4
    Act = mybir.ActivationFunctionType
    Alu = mybir.AluOpType

    nc = bacc.Bacc()

    qT = nc.dram_tensor("qT", [D, L], f32, kind="ExternalInput")
    kT = nc.dram_tensor("kT", [D, L], f32, kind="ExternalInput")
    vT = nc.dram_tensor("vT", [D, L], f32, kind="ExternalInput")
    mb8 = nc.dram_tensor("mb8", [L, L], f8, kind="ExternalInput")
    wq = nc.dram_tensor("wq", [D, 2 * DKC], f32, kind="ExternalInput")
    wk = nc.dram_tensor("wk", [D, 2 * DKC], f32, kind="ExternalInput")
    wv = nc.dram_tensor("wv", [D, DKC], f32, kind="ExternalInput")
    wo = nc.dram_tensor("wo", [DKC, D], f32, kind="ExternalInput")
    bqs = nc.dram_tensor("bqs", [1, 2 * DKC], f32, kind="ExternalInput")
    bks = nc.dram_tensor("bks", [1, 2 * DKC], f32, kind="ExternalInput")
    bvs = nc.dram_tensor("bvs", [1, DKC], f32, kind="ExternalInput")

    attn4 = nc.dram_tensor("attn4", [HPC, L, L], f32, kind="ExternalOutput")
    outp = nc.dram_tensor("outp", [L, D], f32, kind="ExternalOutput")

    ctxs = nc.dram_tensor("ctxs", [HPC * DK, L], f32, kind="Internal")

    with tile.TileContext(nc) as tc:
        with (
            tc.tile_pool(name="const", bufs=1) as cpool,
            tc.tile_pool(name="proj", bufs=1) as jpool,
            tc.tile_pool(name="stream", bufs=3) as xpool,
            tc.tile_pool(name="work", bufs=1) as wkpool,
            tc.tile_pool(name="small", bufs=2) as spool,
            tc.tile_pool(name="psum", bufs=2, space="PSUM") as pspool,
        ):
            # ---------------- constants ----------------
            ident32 = cpool.tile([128, 128], f32)
            make_identity(nc, ident32)
            ident8 = cpool.tile([128, 128], f8)
            nc.vector.tensor_copy(ident8, ident32)
            ones = cpool.tile([1, QB], f32)
            nc.vector.memset(ones, 1.0)
            # ones living at partition 64 (rank-1 matmuls vs row-64 denoms)
            ones64 = cpool.tile([65, 64], f32)
            nc.vector.memset(ones64[64:65, :], 1.0)
            bq_sb = cpool.tile([1, 2 * DKC], f32)
            nc.sync.dma_start(out=bq_sb, in_=bqs[:, :])
            bk_sb = cpool.tile([1, 2 * DKC], f32)
            nc.sync.dma_start(out=bk_sb, in_=bks[:, :])
            bv_sb = cpool.tile([1, DKC], f32)
            nc.sync.dma_start(out=bv_sb, in_=bvs[:, :])

            # mask bias, resident: [128, kc*L + q]
            mb_sb = cpool.tile([128, NKC * L], f8)
            for kc in range(NKC):
                nc.sync.dma_start(
                    out=mb_sb[:, kc * L : (kc + 1) * L],
                    in_=mb8[kc * 128 : (kc + 1) * 128, :],
                )

            # weights resident (undup'd); lhsT slices use stride-0 dup
            wq_sb = cpool.tile([128, 8 * 2 * DKC], f32)
            wk_sb = cpool.tile([128, 8 * 2 * DKC], f32)
            for dc in range(8):
                nc.sync.dma_start(
                    out=wq_sb[:, dc * 512 : dc * 512 + 512],
                    in_=wq[dc * 128 : dc * 128 + 128, :],
                )
                nc.sync.dma_start(
                    out=wk_sb[:, dc * 512 : dc * 512 + 512],
                    in_=wk[dc * 128 : dc * 128 + 128, :],
                )

            # ---------------- projections ----------------
            # qh: per head, [128, L]: rows 0:64 and 64:128 both hold
            # qh_T[h] = (Wq[:,h].T @ q.T + bq)/D  (scaled during evacuation)
            # kh: per head, [128, L//2]: rows 0:64 = kh_T[h][:, :1024],
            #     rows 64:128 = kh_T[h][:, 1024:]
            qh = [jpool.tile([128, L], f32, name=f"qh{h}") for h in range(HPC)]
            kh = [jpool.tile([128, L // 2], f32, name=f"kh{h}") for h in range(HPC)]
            # vh (+ones col): per head [128, NKC*65]; col 64 of each chunk = 1.0
            vha = [jpool.tile([128, NKC * 65], f32, name=f"vh{h}") for h in range(HPC)]
            for h in range(HPC):
                nc.vector.memset(vha[h], 1.0)

            for lb in range(NQB):
                for side in range(2):  # 0 = q, 1 = k
                    src = qT if side == 0 else kT
                    w_sb = wq_sb if side == 0 else wk_sb
                    b_sb = bq_sb if side == 0 else bk_sb
                    xs = []
                    for piece in range(2):
                        x = xpool.tile([128, 4 * QB], f32, tag="xs", name="xs")
                        for i in range(4):
                            dc = piece * 4 + i
                            nc.sync.dma_start(
                                out=x[:, i * QB : i * QB + QB],
                                in_=src[
                                    dc * 128 : dc * 128 + 128, lb * QB : lb * QB + QB
                                ],
                            )
                        xs.append(x)
                    for h in range(HPC):
                        ps = pspool.tile([128, QB], f32, tag="psS")
                        for dc in range(8):
                            nc.tensor.matmul(
                                ps,
                                lhsT=w_sb[
                                    :, dc * 512 + h * 128 : dc * 512 + h * 128 + 128
                                ],
                                rhs=xs[dc // 4][:, (dc % 4) * QB : (dc % 4) * QB + QB],
                                start=(dc == 0),
                                stop=False,
                            )
                        nc.tensor.matmul(
                            ps,
                            lhsT=b_sb[0:1, h * 128 : h * 128 + 128],
                            rhs=ones[0:1, :],
                            start=False,
                            stop=True,
                            tile_position=(0, 0),
                        )
                        if side == 0:
                            nc.scalar.activation(
                                qh[h][:, lb * QB : lb * QB + QB], ps, Act.Copy,
                                scale=1.0 / float(D),
                            )
                        else:
                            half = 0 if lb < 2 else 64
                            nc.scalar.activation(
                                kh[h][
                                    half : half + 64,
                                    (lb % 2) * QB : (lb % 2) * QB + QB,
                                ],
                                ps[half : half + 64, :],
                                Act.Copy,
                            )

            # v projection: out[l, dk] = v @ Wv + bv; stationary = vT chunks
            wv_sb = xpool.tile([128, 8 * DKC], f32, tag="xs", name="wv_sb")
            for dc in range(8):
                nc.sync.dma_start(
                    out=wv_sb[:, dc * DKC : dc * DKC + DKC],
                    in_=wv[dc * 128 : dc * 128 + 128, :],
                )
            for lc in range(NKC):
                vx = xpool.tile([128, 8 * 128], f32, tag="xs", name="vx")
                for dc in range(8):
                    nc.sync.dma_start(
                        out=vx[:, dc * 128 : dc * 128 + 128],
                        in_=vT[dc * 128 : dc * 128 + 128, lc * 128 : lc * 128 + 128],
                    )
                psv = pspool.tile([128, DKC], f32, tag="psS")
                for dc in range(8):
                    nc.tensor.matmul(
                        psv,
                        lhsT=vx[:, dc * 128 : dc * 128 + 128],
                        rhs=wv_sb[:, dc * DKC : dc * DKC + DKC],
                        start=(dc == 0),
                        stop=False,
                    )
                nc.tensor.matmul(
                    psv,
                    lhsT=ones[0:1, 0:128],
                    rhs=bv_sb[0:1, :],
                    start=False,
                    stop=True,
                    tile_position=(0, 0),
                )
                for h in range(HPC):
                    # col 64 of each 65-chunk is the ones column (denominator row)
                    nc.scalar.activation(
                        vha[h][:, lc * 65 : lc * 65 + 64],
                        psv[:, h * DK : h * DK + DK],
                        Act.Copy,
                    )

            # ---------------- attention ----------------
            for h in range(HPC):
                for qb in range(NQB):
                    # exp(S^T) k-chunks for one q-block (single slot, reused)
                    expP_i = wkpool.tile([128, NKC * QB], f32, tag="expP", name="expP")
                    # scores + mask bias + exp + PV accumulation
                    psPV = pspool.tile([65, QB], f32, tag="psPV")
                    for t in range(8):
                        for half, kc in ((0, t), (64, t + 8)):
                            psS = pspool.tile([128, QB], f32, tag="psS")
                            nc.tensor.matmul(
                                psS,
                                lhsT=kh[h][half : half + 64, t * 128 : t * 128 + 128],
                                rhs=qh[h][half : half + 64, qb * QB : qb * QB + QB],
                                start=True,
                                stop=False,
                            )
                            nc.tensor.matmul(
                                psS,
                                lhsT=ident8,
                                rhs=mb_sb[:, kc * L + qb * QB : kc * L + qb * QB + QB],
                                start=False,
                                stop=True,
                            )
                            nc.scalar.activation(
                                expP_i[:, kc * QB : kc * QB + QB], psS, Act.Exp
                            )
                            nc.tensor.matmul(
                                psPV,
                                lhsT=vha[h][:, kc * 65 : kc * 65 + 65],
                                rhs=expP_i[:, kc * QB : kc * QB + QB],
                                start=(kc == 0),
                                stop=(kc == 15),
                            )
                    # denominators (psPV row 64 = ones-row result) -> reciprocals
                    den = spool.tile([65, QB], f32, tag="den", bufs=1)
                    nc.vector.tensor_copy(den[64:65, :], psPV[64:65, :])
                    # per-partition recip for attn tiles: [128, 4] (qg cols)
                    psR = pspool.tile([128, QB], f32, tag="psB")
                    for g in range(4):
                        nc.tensor.matmul(
                            psR[:, g : g + 1],
                            lhsT=den[64:65, g * 128 : g * 128 + 128],
                            rhs=ones64[64:65, 0:1],
                            start=True,
                            stop=True,
                        )
                    recip = spool.tile([128, 4], f32, tag="recip")
                    nc.vector.reciprocal(recip, psR[:, 0:4])
                    # ctx normalize: bcast denom over rows 0..63, recip, multiply
                    psD = pspool.tile([64, QB], f32, tag="psB")
                    nc.tensor.matmul(
                        psD,
                        lhsT=ones64[64:65, 0:64],
                        rhs=den[64:65, :],
                        start=True,
                        stop=True,
                    )
                    recb = spool.tile([64, QB], f32, tag="recb", bufs=1)
                    nc.vector.reciprocal(recb, psD)
                    ctxn = spool.tile([64, QB], f32, tag="ctxn", bufs=1)
                    nc.vector.tensor_tensor(
                        out=ctxn, in0=psPV[0:64, :], in1=recb, op=Alu.mult
                    )
                    nc.sync.dma_start(
                        out=ctxs[h * DK : h * DK + DK, qb * QB : qb * QB + QB],
                        in_=ctxn,
                    )
                    # transpose exp(S^T) -> [q, k], normalize, store
                    for g in range(4):
                        for tt in range(4):
                            psT = pspool.tile([128, QB], f32, tag="psT")
                            for j in range(4):
                                kc = tt * 4 + j
                                nc.tensor.transpose(
                                    psT[:, j * 128 : j * 128 + 128],
                                    expP_i[:, kc * QB + g * 128 : kc * QB + g * 128 + 128],
                                    ident32,
                                )
                            att = spool.tile([128, QB], f32, tag="att")
                            nc.vector.tensor_scalar(
                                att, psT, recip[:, g : g + 1], None, Alu.mult
                            )
                            nc.sync.dma_start(
                                out=attn4[
                                    h,
                                    qb * QB + g * 128 : qb * QB + g * 128 + 128,
                                    tt * QB : tt * QB + QB,
                                ],
                                in_=att,
                            )

            # ---------------- output projection ----------------
            wo_sb = xpool.tile([128, 2 * D], f32, tag="xs", name="wo_sb")
            for c in range(2):
                nc.sync.dma_start(
                    out=wo_sb[:, c * D : c * D + D],
                    in_=wo[c * 128 : c * 128 + 128, :],
                )
            for lc in range(NKC):
                cx = spool.tile([128, 2 * 128], f32, tag="cx")
                for c in range(2):
                    nc.sync.dma_start(
                        out=cx[:, c * 128 : c * 128 + 128],
                        in_=ctxs[c * 128 : c * 128 + 128, lc * 128 : lc * 128 + 128],
                    )
                for db in range(2):
                    psO = pspool.tile([128, QB], f32, tag="psS")
                    for c in range(2):
                        nc.tensor.matmul(
                            psO,
                            lhsT=cx[:, c * 128 : c * 128 + 128],
                            rhs=wo_sb[:, c * D + db * QB : c * D + db * QB + QB],
                            start=(c == 0),
                            stop=(c == 1),
                        )
                    ot = spool.tile([128, QB], f32, tag="ot", bufs=1)
                    nc.scalar.activation(ot, psO, Act.Copy)
                    nc.sync.dma_start(
                        out=outp[lc * 128 : lc * 128 + 128, db * QB : db * QB + QB],
                        in_=ot,
                    )

    nc.compile()
    return nc


_NC = None


def _get_nc():
    global _NC
    if _NC is None:
        _NC = build_bass()
    return _NC


def make_core_inputs(q, k, v, mask, Wq, bq, Wk, bk, Wv, bv, Wo, bo):
    """Build the per-core input dicts (host-side sharding/layout)."""
    per_batch = []
    for b in range(B):
        per_batch.append(
            dict(
                qT=np.ascontiguousarray(np.asarray(q, np.float32)[b].T),
                kT=np.ascontiguousarray(np.asarray(k, np.float32)[b].T),
                vT=np.ascontiguousarray(np.asarray(v, np.float32)[b].T),
                mb8=(
                    (np.asarray(mask[b]).T.astype(np.float32) - 1.0) * (-MASK_BIAS)
                ).astype(F8),
            )
        )
    def dup_cols(w):
        # [D, HPC*DK] -> [D, HPC, 2, DK] -> [D, 2*DKC] (per-head column dup)
        w = np.asarray(w, np.float32).reshape(-1, HPC, DK)
        return np.ascontiguousarray(
            np.broadcast_to(w[:, :, None, :], (w.shape[0], HPC, 2, DK)).reshape(
                w.shape[0], 2 * DKC
            )
        )

    in_maps = []
    for c in range(N_CORES):
        b, g = divmod(c, HPC)
        cols = slice(g * DKC, (g + 1) * DKC)
        in_maps.append(
            dict(
                per_batch[b],
                wq=dup_cols(np.asarray(Wq, np.float32)[:, cols]),
                wk=dup_cols(np.asarray(Wk, np.float32)[:, cols]),
                wv=np.ascontiguousarray(np.asarray(Wv, np.float32)[:, cols]),
                wo=np.ascontiguousarray(np.asarray(Wo, np.float32)[cols, :]),
                bqs=dup_cols(np.asarray(bq, np.float32)[cols].reshape(1, DKC)),
                bks=dup_cols(np.asarray(bk, np.float32)[cols].reshape(1, DKC)),
                bvs=np.ascontiguousarray(np.asarray(bv, np.float32)[cols]).reshape(
                    1, DKC
                ),
            )
        )
    return in_maps


def assemble_outputs(results, bo):
    attn_out = np.empty((H * B, L, L), np.float32)
    out = np.empty((B, L, D), np.float32)
    for b in range(B):
        acc = None
        for g in range(HPC):
            r = results[b * HPC + g]
            for j in range(HPC):
                h = g * HPC + j
                attn_out[h * B + b] = r["attn4"][j]
            acc = r["outp"] if acc is None else acc + r["outp"]
        out[b] = acc + np.asarray(bo, np.float32)[None, :]
    return out, attn_out


def kernel(q, k, v, mask, Wq, bq, Wk, bk, Wv, bv, Wo, bo):
    from concourse.bass_utils import run_bass_kernel_spmd

    nc = _get_nc()
    in_maps = make_core_inputs(q, k, v, mask, Wq, bq, Wk, bk, Wv, bv, Wo, bo)
    res = run_bass_kernel_spmd(nc, in_maps, list(range(N_CORES))).results
    return assemble_outputs(res, bo)


# revision 20
# speedup vs baseline: 1.2486x; 1.2486x over previous
"""Trainium2 Bass kernel for nn_MultiHeadAttention_70540542870156.

Full MHA: out = softmax(mask(QK^T/D)) V Wo + bo, plus the attention matrix
itself as a second output [H*B, L, L].

Sharding (8 cores): core c handles batch b = c//4 and heads
[4*(c%4), 4*(c%4)+4) — i.e. 1 batch x 4 heads per core (dk slice of 256).
Host pre-transposes q/k/v to [D, L] per batch, slices weights, and converts
the mask to an additive fp8 bias (0 / -240) in [k, q] layout.

Device pipeline per core (all-fp32 matmuls):
  - QKV projections on TensorE; q-hat/k-hat written with both SBUF partition
    halves holding each head's rows (via stride-0 broadcast of the weight
    columns), enabling 2-way row-packed score matmuls.
  - Per (head, q-block of 512): S^T tiles [128k x 512q] via packed matmuls,
    mask bias added by an identity matmul streaming the fp8 bias into PSUM,
    single ScalarE exp pass evacuating PSUM, PV matmul with an appended
    ones-row producing context and softmax denominators, TensorE transposes
    of exp(S^T) to [q, k] layout, and a fused VectorE PSUM-evacuate +
    normalize (tensor_scalar by 1/denom per-partition) before the DMA store.
  - Output projection from a DRAM-staged context, normalized upstream.
"""

import os
import sys

import numpy as np

for _p in ("/opt/trn_rl_repo",):
    if _p not in sys.path and os.path.isdir(_p):
        sys.path.insert(0, _p)

import ml_dtypes

B, L, D, H = 2, 2048, 1024, 16
DK = D // H  # 64
HPC = 4  # heads per core
DKC = HPC * DK  # 256
N_CORES = 8
QB = 512  # q-block (moving free dim)
NQB = L // QB  # 4
NKC = L // 128  # 16 k-chunks
F8 = ml_dtypes.float8_e4m3
MASK_BIAS = -240.0  # exp(x - 240) underflows fp32 to 0 for x ~ 0


def build_bass():
    import concourse.bass as bass
    import concourse.mybir as mybir
    import concourse.tile as tile
    from concourse import bacc
    from concourse.masks import make_identity

    f32 = mybir.dt.float32
    f32r = mybir.dt.float32r
    f8 = mybir.dt.float8e4
    Act = mybir.ActivationFunctionType
    Alu = mybir.AluOpType

    nc = bacc.Bacc()

    qT = nc.dram_tensor("qT", [D, L], f32r, kind="ExternalInput")
    kT = nc.dram_tensor("kT", [D, L], f32r, kind="ExternalInput")
    vT = nc.dram_tensor("vT", [D, L], f32r, kind="ExternalInput")
    mb8 = nc.dram_tensor("mb8", [L, L], f8, kind="ExternalInput")
    wq = nc.dram_tensor("wq", [D, 2 * DKC], f32r, kind="ExternalInput")
    wk = nc.dram_tensor("wk", [D, 2 * DKC], f32r, kind="ExternalInput")
    wv = nc.dram_tensor("wv", [D, DKC], f32r, kind="ExternalInput")
    wo = nc.dram_tensor("wo", [DKC, D], f32r, kind="ExternalInput")
    bqs = nc.dram_tensor("bqs", [1, 2 * DKC], f32, kind="ExternalInput")
    bks = nc.dram_tensor("bks", [1, 2 * DKC], f32, kind="ExternalInput")
    bvs = nc.dram_tensor("bvs", [1, DKC], f32, kind="ExternalInput")

    attn4 = nc.dram_tensor("attn4", [HPC, L, L], f32, kind="ExternalOutput")
    outp = nc.dram_tensor("outp", [L, D], f32, kind="ExternalOutput")

    ctxs = nc.dram_tensor("ctxs", [HPC * DK, L], f32r, kind="Internal")

    with tile.TileContext(nc) as tc:
        with (
            tc.tile_pool(name="const", bufs=1) as cpool,
            tc.tile_pool(name="proj", bufs=1) as jpool,
            tc.tile_pool(name="stream", bufs=3) as xpool,
            tc.tile_pool(name="work", bufs=1) as wkpool,
            tc.tile_pool(name="small", bufs=2) as spool,
            tc.tile_pool(name="psum", bufs=2, space="PSUM") as pspool,
        ):
            # ---------------- constants ----------------
            identF = cpool.tile([128, 128], f32)
            make_identity(nc, identF)
            ident32 = cpool.tile([128, 128], f32r)
            nc.vector.tensor_copy(ident32, identF)
            ident8 = cpool.tile([128, 128], f8)
            nc.vector.tensor_copy(ident8, identF)
            # rank-1 helper matmuls stay fp32 (ISA rejects tiny f32r matmuls)
            ones = cpool.tile([1, QB], f32)
            nc.vector.memset(ones, 1.0)
            # ones living at partition 64 (rank-1 matmuls vs row-64 denoms)
            ones64 = cpool.tile([65, 64], f32)
            nc.vector.memset(ones64[64:65, :], 1.0)
            bq_sb = cpool.tile([1, 2 * DKC], f32)
            nc.sync.dma_start(out=bq_sb, in_=bqs[:, :])
            bk_sb = cpool.tile([1, 2 * DKC], f32)
            nc.sync.dma_start(out=bk_sb, in_=bks[:, :])
            bv_sb = cpool.tile([1, DKC], f32)
            nc.sync.dma_start(out=bv_sb, in_=bvs[:, :])

            # mask bias, resident: [128, kc*L + q]
            mb_sb = cpool.tile([128, NKC * L], f8)
            for kc in range(NKC):
                nc.sync.dma_start(
                    out=mb_sb[:, kc * L : (kc + 1) * L],
                    in_=mb8[kc * 128 : (kc + 1) * 128, :],
                )

            # weights resident (undup'd); lhsT slices use stride-0 dup
            wq_sb = cpool.tile([128, 8 * 2 * DKC], f32r)
            wk_sb = cpool.tile([128, 8 * 2 * DKC], f32r)
            for dc in range(8):
                nc.sync.dma_start(
                    out=wq_sb[:, dc * 512 : dc * 512 + 512],
                    in_=wq[dc * 128 : dc * 128 + 128, :],
                )
                nc.sync.dma_start(
                    out=wk_sb[:, dc * 512 : dc * 512 + 512],
                    in_=wk[dc * 128 : dc * 128 + 128, :],
                )

            # ---------------- projections ----------------
            # qh: per head, [128, L]: rows 0:64 and 64:128 both hold
            # qh_T[h] = (Wq[:,h].T @ q.T + bq)/D  (scaled during evacuation)
            # kh: per head, [128, L//2]: rows 0:64 = kh_T[h][:, :1024],
            #     rows 64:128 = kh_T[h][:, 1024:]
            qh = [jpool.tile([128, L], f32r, name=f"qh{h}") for h in range(HPC)]
            kh = [jpool.tile([128, L // 2], f32r, name=f"kh{h}") for h in range(HPC)]
            # vh (+ones col): per head [128, NKC*65]; col 64 of each chunk = 1.0
            vha = [jpool.tile([128, NKC * 65], f32r, name=f"vh{h}") for h in range(HPC)]
            onesC = spool.tile([128, NKC], f32, tag="cx")
            nc.vector.memset(onesC, 1.0)
            for h in range(HPC):
                # only the ones columns (64 mod 65) need initialization
                nc.vector.tensor_copy(
                    vha[h].rearrange("p (c e) -> p c e", e=65)[:, :, 64:65],
                    onesC.unsqueeze(2),
                )

            for lb in range(NQB):
                for side in range(2):  # 0 = q, 1 = k
                    src = qT if side == 0 else kT
                    w_sb = wq_sb if side == 0 else wk_sb
                    b_sb = bq_sb if side == 0 else bk_sb
                    xs = []
                    for piece in range(2):
                        x = xpool.tile([128, 4 * QB], f32r, tag="xs", name="xs")
                        for i in range(4):
                            dc = piece * 4 + i
                            nc.sync.dma_start(
                                out=x[:, i * QB : i * QB + QB],
                                in_=src[
                                    dc * 128 : dc * 128 + 128, lb * QB : lb * QB + QB
                                ],
                            )
                        xs.append(x)
                    for h in range(HPC):
                        ps = pspool.tile([128, QB], f32, tag="psS")
                        for dc in range(8):
                            nc.tensor.matmul(
                                ps,
                                lhsT=w_sb[
                                    :, dc * 512 + h * 128 : dc * 512 + h * 128 + 128
                                ],
                                rhs=xs[dc // 4][:, (dc % 4) * QB : (dc % 4) * QB + QB],
                                start=(dc == 0),
                                stop=False,
                            )
                        nc.tensor.matmul(
                            ps,
                            lhsT=b_sb[0:1, h * 128 : h * 128 + 128],
                            rhs=ones[0:1, :],
                            start=False,
                            stop=True,
                            tile_position=(0, 0),
                        )
                        if side == 0:
                            nc.scalar.activation(
                                qh[h][:, lb * QB : lb * QB + QB], ps, Act.Copy,
                                scale=1.0 / float(D),
                            )
                        else:
                            half = 0 if lb < 2 else 64
                            nc.scalar.activation(
                                kh[h][
                                    half : half + 64,
                                    (lb % 2) * QB : (lb % 2) * QB + QB,
                                ],
                                ps[half : half + 64, :],
                                Act.Copy,
                            )

            # v projection: out[l, dk] = v @ Wv + bv; stationary = vT chunks
            wv_sb = xpool.tile([128, 8 * DKC], f32r, tag="xs", name="wv_sb")
            for dc in range(8):
                nc.sync.dma_start(
                    out=wv_sb[:, dc * DKC : dc * DKC + DKC],
                    in_=wv[dc * 128 : dc * 128 + 128, :],
                )
            for lc in range(NKC):
                vx = xpool.tile([128, 8 * 128], f32r, tag="xs", name="vx")
                for dc in range(8):
                    nc.sync.dma_start(
                        out=vx[:, dc * 128 : dc * 128 + 128],
                        in_=vT[dc * 128 : dc * 128 + 128, lc * 128 : lc * 128 + 128],
                    )
                psv = pspool.tile([128, DKC], f32, tag="psS")
                for dc in range(8):
                    nc.tensor.matmul(
                        psv,
                        lhsT=vx[:, dc * 128 : dc * 128 + 128],
                        rhs=wv_sb[:, dc * DKC : dc * DKC + DKC],
                        start=(dc == 0),
                        stop=False,
                    )
                nc.tensor.matmul(
                    psv,
                    lhsT=ones[0:1, 0:128],
                    rhs=bv_sb[0:1, :],
                    start=False,
                    stop=True,
                    tile_position=(0, 0),
                )
                for h in range(HPC):
                    # col 64 of each 65-chunk is the ones column (denominator row)
                    nc.scalar.activation(
                        vha[h][:, lc * 65 : lc * 65 + 64],
                        psv[:, h * DK : h * DK + DK],
                        Act.Copy,
                    )

            # ---------------- attention ----------------
            for h in range(HPC):
                for qb in range(NQB):
                    # exp(S^T) k-chunks for one q-block (single slot, reused)
                    expP_i = wkpool.tile([128, NKC * QB], f32r, tag="expP", name="expP")
                    # scores + mask bias + exp + PV accumulation
                    psPV = pspool.tile([65, QB], f32, tag="psPV")
                    for t in range(8):
                        for half, kc in ((0, t), (64, t + 8)):
                            psS = pspool.tile([128, QB], f32, tag="psS")
                            nc.tensor.matmul(
                                psS,
                                lhsT=kh[h][half : half + 64, t * 128 : t * 128 + 128],
                                rhs=qh[h][half : half + 64, qb * QB : qb * QB + QB],
                                start=True,
                                stop=False,
                            )
                            nc.tensor.matmul(
                                psS,
                                lhsT=ident8,
                                rhs=mb_sb[:, kc * L + qb * QB : kc * L + qb * QB + QB],
                                start=False,
                                stop=True,
                            )
                            nc.scalar.activation(
                                expP_i[:, kc * QB : kc * QB + QB], psS, Act.Exp
                            )
                            nc.tensor.matmul(
                                psPV,
                                lhsT=vha[h][:, kc * 65 : kc * 65 + 65],
                                rhs=expP_i[:, kc * QB : kc * QB + QB],
                                start=(kc == 0),
                                stop=(kc == 15),
                            )
                    # denominators (psPV row 64 = ones-row result) -> reciprocals
                    den = spool.tile([65, QB], f32, tag="den", bufs=1)
                    nc.vector.tensor_copy(den[64:65, :], psPV[64:65, :])
                    # per-partition recip for attn tiles: [128, 4] (qg cols)
                    psR = pspool.tile([128, QB], f32, tag="psB")
                    for g in range(4):
                        nc.tensor.matmul(
                            psR[:, g : g + 1],
                            lhsT=den[64:65, g * 128 : g * 128 + 128],
                            rhs=ones64[64:65, 0:1],
                            start=True,
                            stop=True,
                        )
                    recip = spool.tile([128, 4], f32, tag="recip")
                    nc.vector.reciprocal(recip, psR[:, 0:4])
                    # ctx normalize: bcast denom over rows 0..63, recip, multiply
                    psD = pspool.tile([64, QB], f32, tag="psB")
                    nc.tensor.matmul(
                        psD,
                        lhsT=ones64[64:65, 0:64],
                        rhs=den[64:65, :],
                        start=True,
                        stop=True,
                    )
                    recb = spool.tile([64, QB], f32, tag="recb", bufs=1)
                    nc.vector.reciprocal(recb, psD)
                    ctxn = spool.tile([64, QB], f32r, tag="ctxn", bufs=1)
                    nc.vector.tensor_tensor(
                        out=ctxn, in0=psPV[0:64, :], in1=recb, op=Alu.mult
                    )
                    nc.sync.dma_start(
                        out=ctxs[h * DK : h * DK + DK, qb * QB : qb * QB + QB],
                        in_=ctxn,
                    )
                    # transpose exp(S^T) -> [q, k], normalize, store
                    for g in range(4):
                        for tt in range(4):
                            psT = pspool.tile([128, QB], f32r, tag="psT")
                            for j in range(4):
                                kc = tt * 4 + j
                                nc.tensor.transpose(
                                    psT[:, j * 128 : j * 128 + 128],
                                    expP_i[:, kc * QB + g * 128 : kc * QB + g * 128 + 128],
                                    ident32,
                                )
                            att = spool.tile([128, QB], f32, tag="att")
                            nc.vector.tensor_scalar(
                                att, psT, recip[:, g : g + 1], None, Alu.mult
                            )
                            nc.sync.dma_start(
                                out=attn4[
                                    h,
                                    qb * QB + g * 128 : qb * QB + g * 128 + 128,
                                    tt * QB : tt * QB + QB,
                                ],
                                in_=att,
                            )

            # ---------------- output projection ----------------
            wo_sb = xpool.tile([128, 2 * D], f32r, tag="xs", name="wo_sb")
            for c in range(2):
                nc.sync.dma_start(
                    out=wo_sb[:, c * D : c * D + D],
                    in_=wo[c * 128 : c * 128 + 128, :],
                )
            for lc in range(NKC):
                cx = spool.tile([128, 2 * 128], f32r, tag="cx")
                for c in range(2):
                    nc.sync.dma_start(
                        out=cx[:, c * 128 : c * 128 + 128],
                        in_=ctxs[c * 128 : c * 128 + 128, lc * 128 : lc * 128 + 128],
                    )
                for db in range(2):
                    psO = pspool.tile([128, QB], f32, tag="psS")
                    for c in range(2):
                        nc.tensor.matmul(
                            psO,
                            lhsT=cx[:, c * 128 : c * 128 + 128],
                            rhs=wo_sb[:, c * D + db * QB : c * D + db * QB + QB],
                            start=(c == 0),
                            stop=(c == 1),
                        )
                    ot = spool.tile([128, QB], f32, tag="ot", bufs=1)
                    nc.scalar.activation(ot, psO, Act.Copy)
                    nc.sync.dma_start(
                        out=outp[lc * 128 : lc * 128 + 128, db * QB : db * QB + QB],
                        in_=ot,
                    )

    nc.compile()
    return nc


_NC = None


def _get_nc():
    global _NC
    if _NC is None:
        _NC = build_bass()
    return _NC


def tf32_round(x):
    """Round fp32 array to tf32 (round-to-nearest-even on the low 13 bits)."""
    x = np.ascontiguousarray(np.asarray(x, np.float32))
    u = x.view(np.uint32)
    u = (u + np.uint32(0xFFF) + ((u >> np.uint32(13)) & np.uint32(1))) & np.uint32(
        0xFFFFE000
    )
    return u.view(np.float32)


def make_core_inputs(q, k, v, mask, Wq, bq, Wk, bk, Wv, bv, Wo, bo):
    """Build the per-core input dicts (host-side sharding/layout)."""
    per_batch = []
    for b in range(B):
        per_batch.append(
            dict(
                qT=tf32_round(np.asarray(q, np.float32)[b].T),
                kT=tf32_round(np.asarray(k, np.float32)[b].T),
                vT=tf32_round(np.asarray(v, np.float32)[b].T),
                mb8=(
                    (np.asarray(mask[b]).T.astype(np.float32) - 1.0) * (-MASK_BIAS)
                ).astype(F8),
            )
        )
    def dup_cols(w):
        # [D, HPC*DK] -> [D, HPC, 2, DK] -> [D, 2*DKC] (per-head column dup)
        w = np.asarray(w, np.float32).reshape(-1, HPC, DK)
        return np.ascontiguousarray(
            np.broadcast_to(w[:, :, None, :], (w.shape[0], HPC, 2, DK)).reshape(
                w.shape[0], 2 * DKC
            )
        )

    in_maps = []
    for c in range(N_CORES):
        b, g = divmod(c, HPC)
        cols = slice(g * DKC, (g + 1) * DKC)
        in_maps.append(
            dict(
                per_batch[b],
                wq=tf32_round(dup_cols(np.asarray(Wq, np.float32)[:, cols])),
                wk=tf32_round(dup_cols(np.asarray(Wk, np.float32)[:, cols])),
                wv=tf32_round(np.asarray(Wv, np.float32)[:, cols]),
                wo=tf32_round(np.asarray(Wo, np.float32)[cols, :]),
                bqs=dup_cols(np.asarray(bq, np.float32)[cols].reshape(1, DKC)),
                bks=dup_cols(np.asarray(bk, np.float32)[cols].reshape(1, DKC)),
                bvs=np.ascontiguousarray(np.asarray(bv, np.float32)[cols].reshape(1, DKC)),
            )
        )
    return in_maps


def assemble_outputs(results, bo):
    attn_out = np.empty((H * B, L, L), np.float32)
    out = np.empty((B, L, D), np.float32)
    for b in range(B):
        acc = None
        for g in range(HPC):
            r = results[b * HPC + g]
            for j in range(HPC):
                h = g * HPC + j
                attn_out[h * B + b] = r["attn4"][j]
            acc = r["outp"] if acc is None else acc + r["outp"]
        out[b] = acc + np.asarray(bo, np.float32)[None, :]
    return out, attn_out


def kernel(q, k, v, mask, Wq, bq, Wk, bk, Wv, bv, Wo, bo):
    from concourse.bass_utils import run_bass_kernel_spmd

    nc = _get_nc()
    in_maps = make_core_inputs(q, k, v, mask, Wq, bq, Wk, bk, Wv, bv, Wo, bo)
    res = run_bass_kernel_spmd(nc, in_maps, list(range(N_CORES))).results
    return assemble_outputs(res, bo)


# revision 21
# speedup vs baseline: 1.5083x; 1.2080x over previous
"""Trainium2 Bass kernel for nn_MultiHeadAttention_70540542870156.

Full MHA: out = softmax(mask(QK^T/D)) V Wo + bo, plus the attention matrix
itself as a second output [H*B, L, L].

Sharding (8 cores): core c handles batch b = c//4 and heads
[4*(c%4), 4*(c%4)+4) — i.e. 1 batch x 4 heads per core (dk slice of 256).
Host pre-transposes q/k/v to [D, L] per batch, slices weights, and converts
the mask to an additive fp8 bias (0 / -240) in [k, q] layout.

Device pipeline per core (all-fp32 matmuls):
  - QKV projections on TensorE; q-hat/k-hat written with both SBUF partition
    halves holding each head's rows (via stride-0 broadcast of the weight
    columns), enabling 2-way row-packed score matmuls.
  - Per (head, q-block of 512): S^T tiles [128k x 512q] via packed matmuls,
    mask bias added by an identity matmul streaming the fp8 bias into PSUM,
    single ScalarE exp pass evacuating PSUM, PV matmul with an appended
    ones-row producing context and softmax denominators, TensorE transposes
    of exp(S^T) to [q, k] layout, and a fused VectorE PSUM-evacuate +
    normalize (tensor_scalar by 1/denom per-partition) before the DMA store.
  - Output projection from a DRAM-staged context, normalized upstream.
"""

import os
import sys

import numpy as np

for _p in ("/opt/trn_rl_repo",):
    if _p not in sys.path and os.path.isdir(_p):
        sys.path.insert(0, _p)

import ml_dtypes

B, L, D, H = 2, 2048, 1024, 16
DK = D // H  # 64
HPC = 4  # heads per core
DKC = HPC * DK  # 256
N_CORES = 8
QB = 512  # q-block (moving free dim)
NQB = L // QB  # 4
NKC = L // 128  # 16 k-chunks
F8 = ml_dtypes.float8_e4m3
MASK_BIAS = -240.0  # exp(x - 240) underflows fp32 to 0 for x ~ 0


def build_bass():
    import concourse.bass as bass
    import concourse.mybir as mybir
    import concourse.tile as tile
    from concourse import bacc
    from concourse.masks import make_identity

    f32 = mybir.dt.float32
    f32r = mybir.dt.float32r
    f8 = mybir.dt.float8e4
    bf16 = mybir.dt.bfloat16
    Act = mybir.ActivationFunctionType
    Alu = mybir.AluOpType

    nc = bacc.Bacc()

    qT = nc.dram_tensor("qT", [D, L], f32r, kind="ExternalInput")
    kT = nc.dram_tensor("kT", [D, L], f32r, kind="ExternalInput")
    vT = nc.dram_tensor("vT", [D, L], f32r, kind="ExternalInput")
    mb8 = nc.dram_tensor("mb8", [L, L], f8, kind="ExternalInput")
    wq = nc.dram_tensor("wq", [D, 2 * DKC], f32r, kind="ExternalInput")
    wk = nc.dram_tensor("wk", [D, 2 * DKC], f32r, kind="ExternalInput")
    wv = nc.dram_tensor("wv", [D, DKC], f32r, kind="ExternalInput")
    wo = nc.dram_tensor("wo", [DKC, D], f32r, kind="ExternalInput")
    bqs = nc.dram_tensor("bqs", [1, 2 * DKC], bf16, kind="ExternalInput")
    bks = nc.dram_tensor("bks", [1, 2 * DKC], bf16, kind="ExternalInput")
    bvs = nc.dram_tensor("bvs", [1, DKC], bf16, kind="ExternalInput")

    attn4 = nc.dram_tensor("attn4", [HPC, L, L], f32, kind="ExternalOutput")
    outp = nc.dram_tensor("outp", [L, D], f32, kind="ExternalOutput")

    ctxs = nc.dram_tensor("ctxs", [HPC * DK, L], f32r, kind="Internal")

    with tile.TileContext(nc) as tc:
        with (
            tc.tile_pool(name="const", bufs=1) as cpool,
            tc.tile_pool(name="proj", bufs=1) as jpool,
            tc.tile_pool(name="stream", bufs=3) as xpool,
            tc.tile_pool(name="work", bufs=1) as wkpool,
            tc.tile_pool(name="small", bufs=2) as spool,
            tc.tile_pool(name="psum", bufs=2, space="PSUM") as pspool,
        ):
            # ---------------- constants ----------------
            identF = cpool.tile([128, 128], f32)
            make_identity(nc, identF)
            ident32 = cpool.tile([128, 128], f32r)
            nc.vector.tensor_copy(ident32, identF)
            ident8 = cpool.tile([128, 128], f8)
            nc.vector.tensor_copy(ident8, identF)
            # rank-1 helper matmuls stay fp32 (ISA rejects tiny f32r matmuls)
            ones = cpool.tile([1, QB], f32)
            nc.vector.memset(ones, 1.0)
            ones_bf = cpool.tile([1, QB], bf16)
            nc.vector.memset(ones_bf, 1.0)
            # ones living at partition 64 (rank-1 matmuls vs row-64 denoms)
            ones64 = cpool.tile([65, 64], f32)
            nc.vector.memset(ones64[64:65, :], 1.0)
            bq_sb = cpool.tile([1, 2 * DKC], bf16)
            nc.sync.dma_start(out=bq_sb, in_=bqs[:, :])
            bk_sb = cpool.tile([1, 2 * DKC], bf16)
            nc.sync.dma_start(out=bk_sb, in_=bks[:, :])
            bv_sb = cpool.tile([1, DKC], bf16)
            nc.sync.dma_start(out=bv_sb, in_=bvs[:, :])

            # mask bias, resident: [128, kc*L + q]
            mb_sb = cpool.tile([128, NKC * L], f8)
            nc.sync.dma_start(
                out=mb_sb.rearrange("p (c q) -> p c q", q=L),
                in_=mb8[:, :].rearrange("(c p) q -> p c q", p=128),
            )

            # weights resident (undup'd); lhsT slices use stride-0 dup
            wq_sb = cpool.tile([128, 8 * 2 * DKC], f32r)
            wk_sb = cpool.tile([128, 8 * 2 * DKC], f32r)
            nc.sync.dma_start(
                out=wq_sb.rearrange("p (c q) -> p c q", q=512),
                in_=wq[:, :].rearrange("(c p) q -> p c q", p=128),
            )
            nc.sync.dma_start(
                out=wk_sb.rearrange("p (c q) -> p c q", q=512),
                in_=wk[:, :].rearrange("(c p) q -> p c q", p=128),
            )

            # ---------------- projections ----------------
            # qh: per head, [128, L]: rows 0:64 and 64:128 both hold
            # qh_T[h] = (Wq[:,h].T @ q.T + bq)/D  (scaled during evacuation)
            # kh: per head, [128, L//2]: rows 0:64 = kh_T[h][:, :1024],
            #     rows 64:128 = kh_T[h][:, 1024:]
            qh = [jpool.tile([128, L], f32r, name=f"qh{h}") for h in range(HPC)]
            kh = [jpool.tile([128, L // 2], f32r, name=f"kh{h}") for h in range(HPC)]
            # vh (+ones col): per head [128, NKC*65]; col 64 of each chunk = 1.0
            vha = [jpool.tile([128, NKC * 65], f32r, name=f"vh{h}") for h in range(HPC)]
            onesC = spool.tile([128, NKC], f32, tag="cx")
            nc.vector.memset(onesC, 1.0)
            for h in range(HPC):
                # only the ones columns (64 mod 65) need initialization
                nc.vector.tensor_copy(
                    vha[h].rearrange("p (c e) -> p c e", e=65)[:, :, 64:65],
                    onesC.unsqueeze(2),
                )

            for lb in range(NQB):
                for side in range(2):  # 0 = q, 1 = k
                    src = qT if side == 0 else kT
                    w_sb = wq_sb if side == 0 else wk_sb
                    b_sb = bq_sb if side == 0 else bk_sb
                    xs = []
                    for piece in range(2):
                        x = xpool.tile([128, 4 * QB], f32r, tag="xs", name="xs")
                        nc.sync.dma_start(
                            out=x.rearrange("p (c q) -> p c q", q=QB),
                            in_=src[
                                piece * 512 : piece * 512 + 512,
                                lb * QB : lb * QB + QB,
                            ].rearrange("(c p) q -> p c q", p=128),
                        )
                        xs.append(x)
                    for h in range(HPC):
                        ps = pspool.tile([128, QB], f32, tag="psS")
                        for dc in range(8):
                            nc.tensor.matmul(
                                ps,
                                lhsT=w_sb[
                                    :, dc * 512 + h * 128 : dc * 512 + h * 128 + 128
                                ],
                                rhs=xs[dc // 4][:, (dc % 4) * QB : (dc % 4) * QB + QB],
                                start=(dc == 0),
                                stop=False,
                            )
                        nc.tensor.matmul(
                            ps,
                            lhsT=b_sb[0:1, h * 128 : h * 128 + 128],
                            rhs=ones_bf[0:1, :],
                            start=False,
                            stop=True,
                            tile_position=(0, 0),
                        )
                        if side == 0:
                            nc.scalar.activation(
                                qh[h][:, lb * QB : lb * QB + QB], ps, Act.Copy,
                                scale=1.0 / float(D),
                            )
                        else:
                            half = 0 if lb < 2 else 64
                            nc.scalar.activation(
                                kh[h][
                                    half : half + 64,
                                    (lb % 2) * QB : (lb % 2) * QB + QB,
                                ],
                                ps[half : half + 64, :],
                                Act.Copy,
                            )

            # v projection: out[l, dk] = v @ Wv + bv; stationary = vT chunks
            wv_sb = xpool.tile([128, 8 * DKC], f32r, tag="xs", name="wv_sb")
            nc.sync.dma_start(
                out=wv_sb.rearrange("p (c q) -> p c q", q=DKC),
                in_=wv[:, :].rearrange("(c p) q -> p c q", p=128),
            )
            for lc in range(NKC):
                vx = xpool.tile([128, 8 * 128], f32r, tag="xs", name="vx")
                nc.sync.dma_start(
                    out=vx.rearrange("p (c q) -> p c q", q=128),
                    in_=vT[:, lc * 128 : lc * 128 + 128].rearrange(
                        "(c p) q -> p c q", p=128
                    ),
                )
                psv = pspool.tile([128, DKC], f32, tag="psS")
                for dc in range(8):
                    nc.tensor.matmul(
                        psv,
                        lhsT=vx[:, dc * 128 : dc * 128 + 128],
                        rhs=wv_sb[:, dc * DKC : dc * DKC + DKC],
                        start=(dc == 0),
                        stop=False,
                    )
                nc.tensor.matmul(
                    psv,
                    lhsT=ones_bf[0:1, 0:128],
                    rhs=bv_sb[0:1, :],
                    start=False,
                    stop=True,
                    tile_position=(0, 0),
                )
                for h in range(HPC):
                    # col 64 of each 65-chunk is the ones column (denominator row)
                    nc.scalar.activation(
                        vha[h][:, lc * 65 : lc * 65 + 64],
                        psv[:, h * DK : h * DK + DK],
                        Act.Copy,
                    )

            # ---------------- attention ----------------
            for h in range(HPC):
                for qb in range(NQB):
                    # exp(S^T) k-chunks in 4 rotating quarter tiles
                    expT = [
                        wkpool.tile(
                            [128, 4 * QB], f32r, tag="expP", bufs=4, name="expP"
                        )
                        for _ in range(4)
                    ]
                    # scores + mask bias + exp + PV accumulation
                    psPV = pspool.tile([65, QB], f32, tag="psPV")
                    for t in range(8):
                        for half, kc in ((0, t), (64, t + 8)):
                            psS = pspool.tile([128, QB], f32, tag="psS")
                            nc.tensor.matmul(
                                psS,
                                lhsT=kh[h][half : half + 64, t * 128 : t * 128 + 128],
                                rhs=qh[h][half : half + 64, qb * QB : qb * QB + QB],
                                start=True,
                                stop=False,
                            )
                            nc.tensor.matmul(
                                psS,
                                lhsT=ident8,
                                rhs=mb_sb[:, kc * L + qb * QB : kc * L + qb * QB + QB],
                                start=False,
                                stop=True,
                            )
                            eslc = expT[kc // 4][
                                :, (kc % 4) * QB : (kc % 4) * QB + QB
                            ]
                            nc.scalar.activation(eslc, psS, Act.Exp)
                            nc.tensor.matmul(
                                psPV,
                                lhsT=vha[h][:, kc * 65 : kc * 65 + 65],
                                rhs=eslc,
                                start=(kc == 0),
                                stop=(kc == 15),
                            )
                    # denominators (psPV row 64 = ones-row result) -> reciprocals
                    den = spool.tile([65, QB], f32, tag="den", bufs=1)
                    nc.vector.tensor_copy(den[64:65, :], psPV[64:65, :])
                    # per-partition recip for attn tiles: [128, 4] (qg cols)
                    psR = pspool.tile([128, QB], f32, tag="psB")
                    for g in range(4):
                        nc.tensor.matmul(
                            psR[:, g : g + 1],
                            lhsT=den[64:65, g * 128 : g * 128 + 128],
                            rhs=ones64[64:65, 0:1],
                            start=True,
                            stop=True,
                        )
                    recip = spool.tile([128, 4], f32, tag="recip")
                    nc.vector.reciprocal(recip, psR[:, 0:4])
                    # ctx normalize: bcast denom over rows 0..63, recip, multiply
                    psD = pspool.tile([64, QB], f32, tag="psB")
                    nc.tensor.matmul(
                        psD,
                        lhsT=ones64[64:65, 0:64],
                        rhs=den[64:65, :],
                        start=True,
                        stop=True,
                    )
                    recb = spool.tile([64, QB], f32, tag="recb", bufs=1)
                    nc.vector.reciprocal(recb, psD)
                    ctxn = spool.tile([64, QB], f32r, tag="ctxn", bufs=1)
                    nc.vector.tensor_tensor(
                        out=ctxn, in0=psPV[0:64, :], in1=recb, op=Alu.mult
                    )
                    nc.sync.dma_start(
                        out=ctxs[h * DK : h * DK + DK, qb * QB : qb * QB + QB],
                        in_=ctxn,
                    )
                    # transpose exp(S^T) -> [q, k], normalize, store
                    for tt in range(4):
                        for g in range(4):
                            psT = pspool.tile([128, QB], f32r, tag="psT")
                            for j in range(4):
                                nc.tensor.transpose(
                                    psT[:, j * 128 : j * 128 + 128],
                                    expT[tt][:, j * QB + g * 128 : j * QB + g * 128 + 128],
                                    ident32,
                                )
                            att = spool.tile([128, QB], f32, tag="att")
                            nc.vector.tensor_scalar(
                                att, psT, recip[:, g : g + 1], None, Alu.mult
                            )
                            nc.sync.dma_start(
                                out=attn4[
                                    h,
                                    qb * QB + g * 128 : qb * QB + g * 128 + 128,
                                    tt * QB : tt * QB + QB,
                                ],
                                in_=att,
                            )

            # ---------------- output projection ----------------
            wo_sb = xpool.tile([128, 2 * D], f32r, tag="xs", name="wo_sb")
            for c in range(2):
                nc.sync.dma_start(
                    out=wo_sb[:, c * D : c * D + D],
                    in_=wo[c * 128 : c * 128 + 128, :],
                )
            for lc in range(NKC):
                cx = spool.tile([128, 2 * 128], f32r, tag="cx")
                for c in range(2):
                    nc.sync.dma_start(
                        out=cx[:, c * 128 : c * 128 + 128],
                        in_=ctxs[c * 128 : c * 128 + 128, lc * 128 : lc * 128 + 128],
                    )
                for db in range(2):
                    psO = pspool.tile([128, QB], f32, tag="psS")
                    for c in range(2):
                        nc.tensor.matmul(
                            psO,
                            lhsT=cx[:, c * 128 : c * 128 + 128],
                            rhs=wo_sb[:, c * D + db * QB : c * D + db * QB + QB],
                            start=(c == 0),
                            stop=(c == 1),
                        )
                    ot = spool.tile([128, QB], f32, tag="ot", bufs=1)
                    nc.scalar.activation(ot, psO, Act.Copy)
                    nc.sync.dma_start(
                        out=outp[lc * 128 : lc * 128 + 128, db * QB : db * QB + QB],
                        in_=ot,
                    )

    nc.compile()
    return nc


_NC = None


def _get_nc():
    global _NC
    if _NC is None:
        _NC = build_bass()
    return _NC


def tf32_round(x):
    """Round fp32 array to tf32 (round-to-nearest-even on the low 13 bits)."""
    x = np.ascontiguousarray(np.asarray(x, np.float32))
    u = x.view(np.uint32)
    u = (u + np.uint32(0xFFF) + ((u >> np.uint32(13)) & np.uint32(1))) & np.uint32(
        0xFFFFE000
    )
    return u.view(np.float32)


def make_core_inputs(q, k, v, mask, Wq, bq, Wk, bk, Wv, bv, Wo, bo):
    """Build the per-core input dicts (host-side sharding/layout)."""
    per_batch = []
    for b in range(B):
        per_batch.append(
            dict(
                qT=tf32_round(np.asarray(q, np.float32)[b].T),
                kT=tf32_round(np.asarray(k, np.float32)[b].T),
                vT=tf32_round(np.asarray(v, np.float32)[b].T),
                mb8=(
                    (np.asarray(mask[b]).T.astype(np.float32) - 1.0) * (-MASK_BIAS)
                ).astype(F8),
            )
        )
    def dup_cols(w):
        # [D, HPC*DK] -> [D, HPC, 2, DK] -> [D, 2*DKC] (per-head column dup)
        w = np.asarray(w, np.float32).reshape(-1, HPC, DK)
        return np.ascontiguousarray(
            np.broadcast_to(w[:, :, None, :], (w.shape[0], HPC, 2, DK)).reshape(
                w.shape[0], 2 * DKC
            )
        )

    in_maps = []
    for c in range(N_CORES):
        b, g = divmod(c, HPC)
        cols = slice(g * DKC, (g + 1) * DKC)
        in_maps.append(
            dict(
                per_batch[b],
                wq=tf32_round(dup_cols(np.asarray(Wq, np.float32)[:, cols])),
                wk=tf32_round(dup_cols(np.asarray(Wk, np.float32)[:, cols])),
                wv=tf32_round(np.asarray(Wv, np.float32)[:, cols]),
                wo=tf32_round(np.asarray(Wo, np.float32)[cols, :]),
                bqs=dup_cols(np.asarray(bq, np.float32)[cols].reshape(1, DKC)).astype(ml_dtypes.bfloat16),
                bks=dup_cols(np.asarray(bk, np.float32)[cols].reshape(1, DKC)).astype(ml_dtypes.bfloat16),
                bvs=np.ascontiguousarray(np.asarray(bv, np.float32)[cols].reshape(1, DKC)).astype(ml_dtypes.bfloat16),
            )
        )
    return in_maps


def assemble_outputs(results, bo):
    attn_out = np.empty((H * B, L, L), np.float32)
    out = np.empty((B, L, D), np.float32)
    for b in range(B):
        acc = None
        for g in range(HPC):
            r = results[b * HPC + g]
            for j in range(HPC):
                h = g * HPC + j
                attn_out[h * B + b] = r["attn4"][j]
            acc = r["outp"] if acc is None else acc + r["outp"]
        out[b] = acc + np.asarray(bo, np.float32)[None, :]
    return out, attn_out


def kernel(q, k, v, mask, Wq, bq, Wk, bk, Wv, bv, Wo, bo):
    from concourse.bass_utils import run_bass_kernel_spmd

    nc = _get_nc()
    in_maps = make_core_inputs(q, k, v, mask, Wq, bq, Wk, bk, Wv, bv, Wo, bo)
    res = run_bass_kernel_spmd(nc, in_maps, list(range(N_CORES))).results
    return assemble_outputs(res, bo)


# revision 22
# speedup vs baseline: 1.5738x; 1.0434x over previous
"""Trainium2 Bass kernel for nn_MultiHeadAttention_70540542870156.

Full MHA: out = softmax(mask(QK^T/D)) V Wo + bo, plus the attention matrix
itself as a second output [H*B, L, L].

Sharding (8 cores): core c handles batch b = c//4 and heads
[4*(c%4), 4*(c%4)+4) — i.e. 1 batch x 4 heads per core (dk slice of 256).
Host pre-transposes q/k/v to [D, L] per batch, slices weights, and converts
the mask to an additive fp8 bias (0 / -240) in [k, q] layout.

Device pipeline per core (all-fp32 matmuls):
  - QKV projections on TensorE; q-hat/k-hat written with both SBUF partition
    halves holding each head's rows (via stride-0 broadcast of the weight
    columns), enabling 2-way row-packed score matmuls.
  - Per (head, q-block of 512): S^T tiles [128k x 512q] via packed matmuls,
    mask bias added by an identity matmul streaming the fp8 bias into PSUM,
    single ScalarE exp pass evacuating PSUM, PV matmul with an appended
    ones-row producing context and softmax denominators, TensorE transposes
    of exp(S^T) to [q, k] layout, and a fused VectorE PSUM-evacuate +
    normalize (tensor_scalar by 1/denom per-partition) before the DMA store.
  - Output projection from a DRAM-staged context, normalized upstream.
"""

import os
import sys

import numpy as np

for _p in ("/opt/trn_rl_repo",):
    if _p not in sys.path and os.path.isdir(_p):
        sys.path.insert(0, _p)

import ml_dtypes

B, L, D, H = 2, 2048, 1024, 16
DK = D // H  # 64
HPC = 4  # heads per core
DKC = HPC * DK  # 256
N_CORES = 8
QB = 512  # q-block (moving free dim)
NQB = L // QB  # 4
NKC = L // 128  # 16 k-chunks
F8 = ml_dtypes.float8_e4m3
MASK_BIAS = -240.0  # exp(x - 240) underflows fp32 to 0 for x ~ 0


def build_bass():
    import concourse.bass as bass
    import concourse.mybir as mybir
    import concourse.tile as tile
    from concourse import bacc
    from concourse.masks import make_identity

    f32 = mybir.dt.float32
    f32r = mybir.dt.float32r
    f8 = mybir.dt.float8e4
    bf16 = mybir.dt.bfloat16
    Act = mybir.ActivationFunctionType
    Alu = mybir.AluOpType

    nc = bacc.Bacc()

    qT = nc.dram_tensor("qT", [D, L], f32r, kind="ExternalInput")
    kT = nc.dram_tensor("kT", [D, L], f32r, kind="ExternalInput")
    vT = nc.dram_tensor("vT", [D, L], f32r, kind="ExternalInput")
    mb8 = nc.dram_tensor("mb8", [L, L], f8, kind="ExternalInput")
    wq = nc.dram_tensor("wq", [D, 2 * DKC], f32r, kind="ExternalInput")
    wk = nc.dram_tensor("wk", [D, 2 * DKC], f32r, kind="ExternalInput")
    wv = nc.dram_tensor("wv", [D, DKC], f32r, kind="ExternalInput")
    wo = nc.dram_tensor("wo", [DKC, D], f32r, kind="ExternalInput")
    bqs = nc.dram_tensor("bqs", [1, 2 * DKC], bf16, kind="ExternalInput")
    bks = nc.dram_tensor("bks", [1, 2 * DKC], bf16, kind="ExternalInput")
    bvs = nc.dram_tensor("bvs", [1, DKC], bf16, kind="ExternalInput")

    attn4 = nc.dram_tensor("attn4", [HPC, L, L], f32, kind="ExternalOutput")
    outp = nc.dram_tensor("outp", [L, D], f32, kind="ExternalOutput")

    ctxs = nc.dram_tensor("ctxs", [HPC * DK, L], f32r, kind="Internal")

    with tile.TileContext(nc) as tc:
        with (
            tc.tile_pool(name="const", bufs=1) as cpool,
            tc.tile_pool(name="proj", bufs=1) as jpool,
            tc.tile_pool(name="stream", bufs=3) as xpool,
            tc.tile_pool(name="work", bufs=1) as wkpool,
            tc.tile_pool(name="small", bufs=2) as spool,
            tc.tile_pool(name="psum", bufs=2, space="PSUM") as pspool,
        ):
            # ---------------- constants ----------------
            identF = cpool.tile([128, 128], f32)
            make_identity(nc, identF)
            ident32 = cpool.tile([128, 128], f32r)
            nc.vector.tensor_copy(ident32, identF)
            ident8 = cpool.tile([128, 128], f8)
            nc.vector.tensor_copy(ident8, identF)
            # rank-1 helper matmuls stay fp32 (ISA rejects tiny f32r matmuls)
            ones = cpool.tile([1, QB], f32)
            nc.vector.memset(ones, 1.0)
            ones_bf = cpool.tile([1, QB], bf16)
            nc.vector.memset(ones_bf, 1.0)
            # ones living at partition 64 (rank-1 matmuls vs row-64 denoms)
            ones64 = cpool.tile([65, 64], f32)
            nc.vector.memset(ones64[64:65, :], 1.0)
            bq_sb = cpool.tile([1, 2 * DKC], bf16)
            nc.sync.dma_start(out=bq_sb, in_=bqs[:, :])
            bk_sb = cpool.tile([1, 2 * DKC], bf16)
            nc.sync.dma_start(out=bk_sb, in_=bks[:, :])
            bv_sb = cpool.tile([1, DKC], bf16)
            nc.sync.dma_start(out=bv_sb, in_=bvs[:, :])

            # mask bias, resident: [128, kc*L + q]
            mb_sb = cpool.tile([128, NKC * L], f8)
            nc.sync.dma_start(
                out=mb_sb.rearrange("p (c q) -> p c q", q=L),
                in_=mb8[:, :].rearrange("(c p) q -> p c q", p=128),
            )

            # weights resident (undup'd); lhsT slices use stride-0 dup
            wq_sb = cpool.tile([128, 8 * 2 * DKC], f32r)
            wk_sb = cpool.tile([128, 8 * 2 * DKC], f32r)
            nc.sync.dma_start(
                out=wq_sb.rearrange("p (c q) -> p c q", q=512),
                in_=wq[:, :].rearrange("(c p) q -> p c q", p=128),
            )
            nc.sync.dma_start(
                out=wk_sb.rearrange("p (c q) -> p c q", q=512),
                in_=wk[:, :].rearrange("(c p) q -> p c q", p=128),
            )

            # ---------------- projections ----------------
            # qh: per head, [128, L]: rows 0:64 and 64:128 both hold
            # qh_T[h] = (Wq[:,h].T @ q.T + bq)/D  (scaled during evacuation)
            # kh: per head, [128, L//2]: rows 0:64 = kh_T[h][:, :1024],
            #     rows 64:128 = kh_T[h][:, 1024:]
            qh = [jpool.tile([128, L], f32r, name=f"qh{h}") for h in range(HPC)]
            kh = [jpool.tile([128, L // 2], f32r, name=f"kh{h}") for h in range(HPC)]
            # vh (+ones col): per head [128, NKC*65]; col 64 of each chunk = 1.0
            vha = [jpool.tile([128, NKC * 65], f32r, name=f"vh{h}") for h in range(HPC)]
            onesC = spool.tile([128, NKC], f32, tag="cx")
            nc.vector.memset(onesC, 1.0)
            for h in range(HPC):
                # only the ones columns (64 mod 65) need initialization
                nc.vector.tensor_copy(
                    vha[h].rearrange("p (c e) -> p c e", e=65)[:, :, 64:65],
                    onesC.unsqueeze(2),
                )

            for lb in range(NQB):
                for side in range(2):  # 0 = q, 1 = k
                    src = qT if side == 0 else kT
                    w_sb = wq_sb if side == 0 else wk_sb
                    b_sb = bq_sb if side == 0 else bk_sb
                    xs = []
                    for piece in range(2):
                        x = xpool.tile([128, 4 * QB], f32r, tag="xs", name="xs")
                        nc.sync.dma_start(
                            out=x.rearrange("p (c q) -> p c q", q=QB),
                            in_=src[
                                piece * 512 : piece * 512 + 512,
                                lb * QB : lb * QB + QB,
                            ].rearrange("(c p) q -> p c q", p=128),
                        )
                        xs.append(x)
                    for h in range(HPC):
                        ps = pspool.tile([128, QB], f32, tag="psS", bufs=3)
                        for dc in range(8):
                            nc.tensor.matmul(
                                ps,
                                lhsT=w_sb[
                                    :, dc * 512 + h * 128 : dc * 512 + h * 128 + 128
                                ],
                                rhs=xs[dc // 4][:, (dc % 4) * QB : (dc % 4) * QB + QB],
                                start=(dc == 0),
                                stop=False,
                            )
                        nc.tensor.matmul(
                            ps,
                            lhsT=b_sb[0:1, h * 128 : h * 128 + 128],
                            rhs=ones_bf[0:1, :],
                            start=False,
                            stop=True,
                            tile_position=(0, 0),
                        )
                        if side == 0:
                            nc.scalar.activation(
                                qh[h][:, lb * QB : lb * QB + QB], ps, Act.Copy,
                                scale=1.0 / float(D),
                            )
                        else:
                            half = 0 if lb < 2 else 64
                            nc.scalar.activation(
                                kh[h][
                                    half : half + 64,
                                    (lb % 2) * QB : (lb % 2) * QB + QB,
                                ],
                                ps[half : half + 64, :],
                                Act.Copy,
                            )

            # v projection: out[l, dk] = v @ Wv + bv; stationary = vT chunks
            wv_sb = xpool.tile([128, 8 * DKC], f32r, tag="xs", name="wv_sb")
            nc.sync.dma_start(
                out=wv_sb.rearrange("p (c q) -> p c q", q=DKC),
                in_=wv[:, :].rearrange("(c p) q -> p c q", p=128),
            )
            for lc in range(NKC):
                vx = xpool.tile([128, 8 * 128], f32r, tag="xs", name="vx")
                nc.sync.dma_start(
                    out=vx.rearrange("p (c q) -> p c q", q=128),
                    in_=vT[:, lc * 128 : lc * 128 + 128].rearrange(
                        "(c p) q -> p c q", p=128
                    ),
                )
                psv = pspool.tile([128, DKC], f32, tag="psS", bufs=3)
                for dc in range(8):
                    nc.tensor.matmul(
                        psv,
                        lhsT=vx[:, dc * 128 : dc * 128 + 128],
                        rhs=wv_sb[:, dc * DKC : dc * DKC + DKC],
                        start=(dc == 0),
                        stop=False,
                    )
                nc.tensor.matmul(
                    psv,
                    lhsT=ones_bf[0:1, 0:128],
                    rhs=bv_sb[0:1, :],
                    start=False,
                    stop=True,
                    tile_position=(0, 0),
                )
                for h in range(HPC):
                    # col 64 of each 65-chunk is the ones column (denominator row)
                    nc.scalar.activation(
                        vha[h][:, lc * 65 : lc * 65 + 64],
                        psv[:, h * DK : h * DK + DK],
                        Act.Copy,
                    )

            # ---------------- attention ----------------
            for h in range(HPC):
                for qb in range(NQB):
                    # exp(S^T) k-chunks in 4 rotating quarter tiles
                    expT = [
                        wkpool.tile(
                            [128, 4 * QB], f32r, tag="expP", bufs=4, name="expP"
                        )
                        for _ in range(4)
                    ]
                    # scores + mask bias + exp + PV accumulation
                    psPV = pspool.tile([65, QB], f32, tag="psPV", bufs=1)
                    for t in range(8):
                        for half, kc in ((0, t), (64, t + 8)):
                            psS = pspool.tile([128, QB], f32, tag="psS", bufs=3)
                            nc.tensor.matmul(
                                psS,
                                lhsT=kh[h][half : half + 64, t * 128 : t * 128 + 128],
                                rhs=qh[h][half : half + 64, qb * QB : qb * QB + QB],
                                start=True,
                                stop=False,
                            )
                            nc.tensor.matmul(
                                psS,
                                lhsT=ident8,
                                rhs=mb_sb[:, kc * L + qb * QB : kc * L + qb * QB + QB],
                                start=False,
                                stop=True,
                            )
                            eslc = expT[kc // 4][
                                :, (kc % 4) * QB : (kc % 4) * QB + QB
                            ]
                            nc.scalar.activation(eslc, psS, Act.Exp)
                            nc.tensor.matmul(
                                psPV,
                                lhsT=vha[h][:, kc * 65 : kc * 65 + 65],
                                rhs=eslc,
                                start=(kc == 0),
                                stop=(kc == 15),
                            )
                    # denominators (psPV row 64 = ones-row result) -> reciprocals
                    den = spool.tile([65, QB], f32, tag="den", bufs=1)
                    nc.scalar.activation(den[64:65, :], psPV[64:65, :], Act.Copy)
                    # per-partition recip for attn tiles: [128, 4] (qg cols)
                    psR = pspool.tile([128, QB], f32, tag="psB")
                    for g in range(4):
                        nc.tensor.matmul(
                            psR[:, g : g + 1],
                            lhsT=den[64:65, g * 128 : g * 128 + 128],
                            rhs=ones64[64:65, 0:1],
                            start=True,
                            stop=True,
                        )
                    recip = spool.tile([128, 4], f32, tag="recip")
                    nc.vector.reciprocal(recip, psR[:, 0:4])
                    # ctx normalize: bcast denom over rows 0..63, recip, multiply
                    psD = pspool.tile([64, QB], f32, tag="psB")
                    nc.tensor.matmul(
                        psD,
                        lhsT=ones64[64:65, 0:64],
                        rhs=den[64:65, :],
                        start=True,
                        stop=True,
                    )
                    recb = spool.tile([64, QB], f32, tag="recb", bufs=1)
                    nc.vector.reciprocal(recb, psD)
                    ctxn = spool.tile([64, QB], f32r, tag="ctxn", bufs=1)
                    nc.vector.tensor_tensor(
                        out=ctxn, in0=psPV[0:64, :], in1=recb, op=Alu.mult
                    )
                    nc.sync.dma_start(
                        out=ctxs[h * DK : h * DK + DK, qb * QB : qb * QB + QB],
                        in_=ctxn,
                    )
                    # transpose exp(S^T) -> [q, k], normalize, store
                    for tt in range(4):
                        for g in range(4):
                            psT = pspool.tile([128, QB], f32r, tag="psT")
                            for j in range(4):
                                nc.tensor.transpose(
                                    psT[:, j * 128 : j * 128 + 128],
                                    expT[tt][:, j * QB + g * 128 : j * QB + g * 128 + 128],
                                    ident32,
                                )
                            att = spool.tile([128, QB], f32, tag="att")
                            if tt % 2 == 0:
                                nc.vector.tensor_scalar(
                                    att, psT, recip[:, g : g + 1], None, Alu.mult
                                )
                            else:
                                nc.scalar.activation(
                                    att, psT, Act.Copy, scale=recip[:, g : g + 1]
                                )
                            nc.sync.dma_start(
                                out=attn4[
                                    h,
                                    qb * QB + g * 128 : qb * QB + g * 128 + 128,
                                    tt * QB : tt * QB + QB,
                                ],
                                in_=att,
                            )

            # ---------------- output projection ----------------
            wo_sb = xpool.tile([128, 2 * D], f32r, tag="xs", name="wo_sb")
            for c in range(2):
                nc.sync.dma_start(
                    out=wo_sb[:, c * D : c * D + D],
                    in_=wo[c * 128 : c * 128 + 128, :],
                )
            for lc in range(NKC):
                cx = spool.tile([128, 2 * 128], f32r, tag="cx")
                for c in range(2):
                    nc.sync.dma_start(
                        out=cx[:, c * 128 : c * 128 + 128],
                        in_=ctxs[c * 128 : c * 128 + 128, lc * 128 : lc * 128 + 128],
                    )
                for db in range(2):
                    psO = pspool.tile([128, QB], f32, tag="psS", bufs=3)
                    for c in range(2):
                        nc.tensor.matmul(
                            psO,
                            lhsT=cx[:, c * 128 : c * 128 + 128],
                            rhs=wo_sb[:, c * D + db * QB : c * D + db * QB + QB],
                            start=(c == 0),
                            stop=(c == 1),
                        )
                    ot = spool.tile([128, QB], f32, tag="ot", bufs=1)
                    nc.scalar.activation(ot, psO, Act.Copy)
                    nc.sync.dma_start(
                        out=outp[lc * 128 : lc * 128 + 128, db * QB : db * QB + QB],
                        in_=ot,
                    )

    nc.compile()
    return nc


_NC = None


def _get_nc():
    global _NC
    if _NC is None:
        _NC = build_bass()
    return _NC


def tf32_round(x):
    """Round fp32 array to tf32 (round-to-nearest-even on the low 13 bits)."""
    x = np.ascontiguousarray(np.asarray(x, np.float32))
    u = x.view(np.uint32)
    u = (u + np.uint32(0xFFF) + ((u >> np.uint32(13)) & np.uint32(1))) & np.uint32(
        0xFFFFE000
    )
    return u.view(np.float32)


def make_core_inputs(q, k, v, mask, Wq, bq, Wk, bk, Wv, bv, Wo, bo):
    """Build the per-core input dicts (host-side sharding/layout)."""
    per_batch = []
    for b in range(B):
        per_batch.append(
            dict(
                qT=tf32_round(np.asarray(q, np.float32)[b].T),
                kT=tf32_round(np.asarray(k, np.float32)[b].T),
                vT=tf32_round(np.asarray(v, np.float32)[b].T),
                mb8=(
                    (np.asarray(mask[b]).T.astype(np.float32) - 1.0) * (-MASK_BIAS)
                ).astype(F8),
            )
        )
    def dup_cols(w):
        # [D, HPC*DK] -> [D, HPC, 2, DK] -> [D, 2*DKC] (per-head column dup)
        w = np.asarray(w, np.float32).reshape(-1, HPC, DK)
        return np.ascontiguousarray(
            np.broadcast_to(w[:, :, None, :], (w.shape[0], HPC, 2, DK)).reshape(
                w.shape[0], 2 * DKC
            )
        )

    in_maps = []
    for c in range(N_CORES):
        b, g = divmod(c, HPC)
        cols = slice(g * DKC, (g + 1) * DKC)
        in_maps.append(
            dict(
                per_batch[b],
                wq=tf32_round(dup_cols(np.asarray(Wq, np.float32)[:, cols])),
                wk=tf32_round(dup_cols(np.asarray(Wk, np.float32)[:, cols])),
                wv=tf32_round(np.asarray(Wv, np.float32)[:, cols]),
                wo=tf32_round(np.asarray(Wo, np.float32)[cols, :]),
                bqs=dup_cols(np.asarray(bq, np.float32)[cols].reshape(1, DKC)).astype(ml_dtypes.bfloat16),
                bks=dup_cols(np.asarray(bk, np.float32)[cols].reshape(1, DKC)).astype(ml_dtypes.bfloat16),
                bvs=np.ascontiguousarray(np.asarray(bv, np.float32)[cols].reshape(1, DKC)).astype(ml_dtypes.bfloat16),
            )
        )
    return in_maps


def assemble_outputs(results, bo):
    attn_out = np.empty((H * B, L, L), np.float32)
    out = np.empty((B, L, D), np.float32)
    for b in range(B):
        acc = None
        for g in range(HPC):
            r = results[b * HPC + g]
            for j in range(HPC):
                h = g * HPC + j
                attn_out[h * B + b] = r["attn4"][j]
            acc = r["outp"] if acc is None else acc + r["outp"]
        out[b] = acc + np.asarray(bo, np.float32)[None, :]
    return out, attn_out


def kernel(q, k, v, mask, Wq, bq, Wk, bk, Wv, bv, Wo, bo):
    from concourse.bass_utils import run_bass_kernel_spmd

    nc = _get_nc()
    in_maps = make_core_inputs(q, k, v, mask, Wq, bq, Wk, bk, Wv, bv, Wo, bo)
    res = run_bass_kernel_spmd(nc, in_maps, list(range(N_CORES))).results
    return assemble_outputs(res, bo)
